# revision 2
# baseline (speedup 1.0000x reference)
"""Trainium2 Bass kernel for DecGridDeepVPN (gnn_message_passing), 8-core SPMD.

Math (per batch row b, agents n=0..19):
  nsc[b]  = action_count[b].reshape(405) @ INFLOW            # [81]
  ir[b]   = sum_d min(nsc[b,d], demand[b,d])                 # scalar
  feat    = [nsc | onehot81(loc[b,n]) | la[b,n] | onehot20(n)]   # 187
  h0 = relu(feat @ W0); h1 = relu(h0 @ W1); out[b,n] = h1@W2 + b2 + ir[b]

Sharding: pure data-parallel over batch (4096 rows/core). Inside each core
everything runs feature-major (features on partitions, batch streaming on the
free axis). Columns of the MLP stage are (b, g) pairs with g = n//4; the four
agent classes j = n%4 occupy four 32-row partition blocks, so layers 0/1/2
run as single block-diagonal matmuls with K=M=128 (the PE touches each column
once per layer instead of four times).

Layer-0 decomposition (W0 = [W0a | W0b | W0c | W0d] over the feature concat):
  zbase = W0a.T @ nscT        broadcast over agents via a 0-step rhs AP
  zg    = W0b[loc]            embedding-table lookup; this toolchain exposes
                              no usable device gather (gpsimd custom ISA ops
                              don't compile here), so the lookup is done as
                              host-side layout prep and streamed in bf16
  la/agent one-hot terms      one K=25 block-diagonal matmul
ir (+b2) is reduced on device (ones-vector matmul) and added exactly (fp32)
on the host during unsharding.

All streamed operands are bf16 (inputs are O(1) uniforms; worst-case output
error ~1e-3 relative); PSUM accumulation and the outputs are fp32.
"""

import dataclasses as _dc

import numpy as np
import ml_dtypes

import concourse.bass as bass
import concourse.mybir as mybir
from concourse.bass_utils import run_bass_kernel_spmd

BF16 = ml_dtypes.bfloat16
F32 = mybir.dt.float32
BF = mybir.dt.bfloat16

S = 81          # grid states
GRID = 9
N_AG = 20       # agents
A = 5           # actions
B = 32768
NCORES = 8
BC = B // NCORES      # 4096 batch rows per core
NBB = 8               # b-blocks per core (phase N)
BB = BC // NBB        # 512
NG = BC * 5           # 20480 (b, g) columns per core
CB = 510              # col-block: multiple of 5, fits one PSUM bank
NCB = (NG + CB - 1) // CB   # 41 (last block = 80 cols)
NGP = NCB * CB        # padded column count


def _grid_inflow():
    moves = [(0, 0), (-1, 0), (1, 0), (0, -1), (0, 1)]
    inflow = np.zeros((S * A, S), np.float32)
    for s in range(S):
        r, c = divmod(s, GRID)
        for a, (dr, dc) in enumerate(moves):
            nr, nc_ = r + dr, c + dc
            d = nr * GRID + nc_ if (0 <= nr < GRID and 0 <= nc_ < GRID) else s
            inflow[s * A + a, d] = 1.0
    return inflow


def _cw(k):
    return min(CB, NG - k * CB)


def _build(reps=1):
    nc = bass.Bass()

    acT = nc.declare_dram_parameter("acT", [4, NBB, 128, BB], BF, isOutput=False)
    demT = nc.declare_dram_parameter("demT", [NBB, S, BB], BF, isOutput=False)
    zgb = nc.declare_dram_parameter("zgb", [NCB, 128, CB], BF, isOutput=False)
    rhs0 = nc.declare_dram_parameter("rhs0", [NCB, 25, CB], BF, isOutput=False)
    wIN = nc.declare_dram_parameter("wIN", [4, 128, S], BF, isOutput=False)
    wA = nc.declare_dram_parameter("wA", [S, 32], BF, isOutput=False)
    w0le = nc.declare_dram_parameter("w0le", [25, 128], BF, isOutput=False)
    wREP = nc.declare_dram_parameter("wREP", [32, 128], BF, isOutput=False)
    w1 = nc.declare_dram_parameter("w1", [128, 128], BF, isOutput=False)
    w2i = nc.declare_dram_parameter("w2i", [128, 4], BF, isOutput=False)
    ones81 = nc.declare_dram_parameter("ones81", [S, 1], BF, isOutput=False)
    out4 = nc.declare_dram_parameter("out4", [NCB, 4, CB], F32, isOutput=True)
    irO = nc.declare_dram_parameter("irO", [1, BC], F32, isOutput=True)

    NW = 10  # weight DMAs

    from contextlib import ExitStack
    ctx = ExitStack()
    with ctx:
        s_ac0 = ctx.enter_context(nc.sbuf_tensor([128, 4 * BB], BF))
        s_ac1 = ctx.enter_context(nc.sbuf_tensor([128, 4 * BB], BF))
        s_ac2 = ctx.enter_context(nc.sbuf_tensor([128, 4 * BB], BF))
        s_ac3 = ctx.enter_context(nc.sbuf_tensor([128, 4 * BB], BF))
        s_dem = ctx.enter_context(nc.sbuf_tensor([S, BC], BF))
        s_nscT = ctx.enter_context(nc.sbuf_tensor([S, BC], BF))
        s_srv = ctx.enter_context(nc.sbuf_tensor([S, BC], BF))
        s_zb = ctx.enter_context(nc.sbuf_tensor([32, BC], BF))
        s_ir = ctx.enter_context(nc.sbuf_tensor([1, BC], F32))
        s_zg = ctx.enter_context(nc.sbuf_tensor([128, 2 * CB], BF))
        s_r0 = ctx.enter_context(nc.sbuf_tensor([25, 2 * CB], BF))
        s_h0p = ctx.enter_context(nc.sbuf_tensor([128, 2 * CB], BF))
        s_h0 = ctx.enter_context(nc.sbuf_tensor([128, 2 * CB], BF))
        s_h1 = ctx.enter_context(nc.sbuf_tensor([128, 2 * CB], BF))
        s_o4 = ctx.enter_context(nc.sbuf_tensor([4, 4 * CB], F32))
        s_wIN0 = ctx.enter_context(nc.sbuf_tensor([128, S], BF))
        s_wIN1 = ctx.enter_context(nc.sbuf_tensor([128, S], BF))
        s_wIN2 = ctx.enter_context(nc.sbuf_tensor([128, S], BF))
        s_wIN3 = ctx.enter_context(nc.sbuf_tensor([128, S], BF))
        s_wA = ctx.enter_context(nc.sbuf_tensor([S, 32], BF))
        s_w0le = ctx.enter_context(nc.sbuf_tensor([25, 128], BF))
        s_wREP = ctx.enter_context(nc.sbuf_tensor([32, 128], BF))
        s_w1 = ctx.enter_context(nc.sbuf_tensor([128, 128], BF))
        s_w2i = ctx.enter_context(nc.sbuf_tensor([128, 4], BF))
        s_ones = ctx.enter_context(nc.sbuf_tensor([S, 1], BF))
        psum = ctx.enter_context(nc.psum_tensor([128, 4096], F32))
        d_w = ctx.enter_context(nc.semaphore())
        d_ac = [ctx.enter_context(nc.semaphore(f'd_ac{_i}')) for _i in range(4)]
        d_dem = ctx.enter_context(nc.semaphore())
        d_m = [ctx.enter_context(nc.semaphore(f'd_m{_i}')) for _i in range(2)]
        d_out = [ctx.enter_context(nc.semaphore(f'd_out{_i}')) for _i in range(4)]
        d_ir = ctx.enter_context(nc.semaphore())
        t_nsc = ctx.enter_context(nc.semaphore())
        t_zbir = ctx.enter_context(nc.semaphore())
        t_h0 = ctx.enter_context(nc.semaphore())
        t_h1 = ctx.enter_context(nc.semaphore())
        t_o = ctx.enter_context(nc.semaphore())
        v_srv = ctx.enter_context(nc.semaphore())
        v_h0 = ctx.enter_context(nc.semaphore())
        sc_nsc = ctx.enter_context(nc.semaphore())
        sc_zbir = ctx.enter_context(nc.semaphore())
        sc_h0 = ctx.enter_context(nc.semaphore())
        sc_h1 = ctx.enter_context(nc.semaphore())
        sc_o = ctx.enter_context(nc.semaphore())
        block = ctx.enter_context(nc.Block())
        s_acs = [s_ac0, s_ac1, s_ac2, s_ac3]
        s_wINs = [s_wIN0, s_wIN1, s_wIN2, s_wIN3]

        def p_nsc(i):
            return psum[0:S, (i % 2) * 512:(i % 2) * 512 + BB]

        def p_misc(i):
            return psum[:, 1024 + (i % 2) * 512: 1024 + (i % 2) * 512 + BB]

        def p_h0(gk):
            return psum[:, (gk % 2) * 512:(gk % 2) * 512 + _cw(gk % NCB)]

        def p_h1(gk):
            return psum[:, 1024 + (gk % 2) * 512:1024 + (gk % 2) * 512 + _cw(gk % NCB)]

        def p_o(gk):
            return psum[0:4, 2048 + (gk % 4) * 512:2048 + (gk % 4) * 512 + _cw(gk % NCB)]

        @block.sync
        def _(sync):
            for i in range(4):
                sync.dma_start(s_wINs[i][:, :], wIN[i]).then_inc(d_w, 16)
            sync.dma_start(s_wA[:, :], wA[:, :]).then_inc(d_w, 16)
            sync.dma_start(s_w0le[:, :], w0le[:, :]).then_inc(d_w, 16)
            sync.dma_start(s_wREP[:, :], wREP[:, :]).then_inc(d_w, 16)
            sync.dma_start(s_w1[:, :], w1[:, :]).then_inc(d_w, 16)
            sync.dma_start(s_w2i[:, :], w2i[:, :]).then_inc(d_w, 16)
            sync.dma_start(s_ones[:, :], ones81[:, :]).then_inc(d_w, 16)
            for r in range(reps):
                if r >= 1:
                    sync.wait_ge(v_srv, NBB * r)  # s_dem consumed by prev rep
                for i in range(NBB):
                    sync.dma_start(
                        s_dem[:, i * BB:(i + 1) * BB], demT[i]
                    ).then_inc(d_dem, 16)
            for gi in range(reps * NBB):
                i = gi % NBB
                if gi >= 4:
                    sync.wait_ge(t_nsc, gi - 3)  # ac buffer slot free
                sl = slice((gi % 4) * BB, (gi % 4) * BB + BB)
                for c in range(4):
                    sync.dma_start(s_acs[c][:, sl], acT[c, i]).then_inc(
                        d_ac[gi % 4], 16)
            for gk in range(reps * NCB):
                k = gk % NCB
                cw = _cw(k)
                sl = slice((gk % 2) * CB, (gk % 2) * CB + cw)
                if gk >= 2:
                    sync.wait_ge(v_h0, 2 * (gk - 1))  # zg/r0 slot consumed
                sync.dma_start(s_zg[:, sl], zgb[k, :, 0:cw]).then_inc(
                    d_m[gk % 2], 16)
                sync.dma_start(s_r0[:, sl], rhs0[k, :, 0:cw]).then_inc(
                    d_m[gk % 2], 16)

        @block.gpsimd
        def _(gpsimd):
            for gk in range(reps * NCB):
                k = gk % NCB
                gpsimd.wait_ge(sc_o, gk + 1)
                cw = _cw(k)
                sl = slice((gk % 4) * CB, (gk % 4) * CB + cw)
                gpsimd.dma_start(out4[k, :, 0:cw], s_o4[:, sl]).then_inc(
                    d_out[gk % 4], 16)
                if k == NCB - 1:
                    r = gk // NCB
                    gpsimd.wait_ge(sc_zbir, 2 * NBB * (r + 1))
                    gpsimd.dma_start(irO[:, :], s_ir[:, :]).then_inc(d_ir, 16)

        @block.tensor
        def _(tensor):
            tensor.wait_ge(d_w, NW * 16)
            for r in range(reps):
                base_i = r * NBB
                base_k = r * NCB
                # ---- phase N: nscT ----
                for i in range(NBB):
                    gi = base_i + i
                    tensor.wait_ge(d_ac[gi % 4], (gi // 4 + 1) * 64)
                    if r >= 1 and i < 2:
                        tensor.wait_ge(sc_h0, NCB * r)  # banks 0-1 freed of h0
                    if gi >= 2:
                        tensor.wait_ge(sc_nsc, gi - 1)  # psum buf free
                    sl = slice((gi % 4) * BB, (gi % 4) * BB + BB)
                    for c in range(4):
                        mm = nc.tensor.matmul(
                            p_nsc(gi), s_wINs[c][:, :], s_acs[c][:, sl],
                            start=(c == 0), stop=(c == 3),
                        )
                    mm.then_inc(t_nsc, 1)
                # ---- phase N: ir reduce + zbase ----
                for i in range(NBB):
                    gi = base_i + i
                    bsl = slice(i * BB, i * BB + BB)
                    pm = p_misc(gi)
                    tensor.wait_ge(v_srv, gi + 1)
                    if r >= 1 and i < 2:
                        tensor.wait_ge(sc_h1, NCB * r)  # banks 2-3 freed of h1
                    if gi >= 2:
                        tensor.wait_ge(sc_zbir, 2 * (gi - 1))  # psum buf free
                    nc.tensor.matmul(
                        pm[0:1, :], s_ones[:, :], s_srv[:, bsl],
                        start=True, stop=True,
                    )
                    tensor.wait_ge(sc_nsc, gi + 1)
                    nc.tensor.matmul(
                        pm[32:64, :], s_wA[:, :], s_nscT[:, bsl],
                        start=True, stop=True,
                    ).then_inc(t_zbir, 1)
                # ---- phase M, software-pipelined ----
                tensor.wait_ge(sc_zbir, 2 * NBB * (r + 1))
                tensor.wait_ge(sc_nsc, NBB * (r + 1))  # banks 0-1 freed of nsc
                for kk in range(NCB + 2):
                    if kk < NCB:
                        k = kk
                        gk = base_k + k
                        cw = _cw(k)
                        sl = slice((gk % 2) * CB, (gk % 2) * CB + cw)
                        tensor.wait_ge(d_m[gk % 2], (gk // 2 + 1) * 32)
                        if gk >= 2:
                            tensor.wait_ge(sc_h0, gk - 1)  # p_h0 buf free
                        nc.tensor.matmul(
                            p_h0(gk), s_w0le[:, :], s_r0[:, sl],
                            start=True, stop=False,
                        )
                        b0 = (k * CB) // 5
                        nb = cw // 5
                        zb_ap = s_zb[:, b0:b0 + nb]
                        zb_ap = _dc.replace(
                            zb_ap, ap=[zb_ap.ap[0], [1, nb], [0, 5]])
                        nc.tensor.matmul(
                            p_h0(gk), s_wREP[:, :], zb_ap,
                            start=False, stop=True,
                        ).then_inc(t_h0, 1)
                    if 1 <= kk and kk - 1 < NCB:
                        k = kk - 1
                        gk = base_k + k
                        cw = _cw(k)
                        sl = slice((gk % 2) * CB, (gk % 2) * CB + cw)
                        tensor.wait_ge(v_h0, 2 * gk + 2)  # h0 ready
                        if gk >= 2:
                            tensor.wait_ge(sc_h1, gk - 1)  # p_h1 buf free
                        nc.tensor.matmul(
                            p_h1(gk), s_w1[:, :], s_h0[:, sl],
                            start=True, stop=True,
                        ).then_inc(t_h1, 1)
                    if 2 <= kk and kk - 2 < NCB:
                        k = kk - 2
                        gk = base_k + k
                        cw = _cw(k)
                        sl = slice((gk % 2) * CB, (gk % 2) * CB + cw)
                        tensor.wait_ge(sc_h1, gk + 1)
                        if gk >= 4:
                            tensor.wait_ge(d_out[gk % 4], ((gk - 4) // 4 + 1) * 16)
                        nc.tensor.matmul(
                            p_o(gk), s_w2i[:, :], s_h1[:, sl],
                            start=True, stop=True,
                        ).then_inc(t_o, 1)

        @block.vector
        def _(vector):
            for gi in range(reps * NBB):
                i = gi % NBB
                r = gi // NBB
                bsl = slice(i * BB, i * BB + BB)
                if i == 0:
                    vector.wait_ge(d_dem, NBB * 16 * (r + 1))
                vector.wait_ge(sc_nsc, gi + 1)
                if gi >= NBB:
                    vector.wait_ge(t_zbir, gi - NBB + 1)  # s_srv consumed
                nc.vector.tensor_tensor(
                    s_srv[:, bsl], s_nscT[:, bsl], s_dem[:, bsl],
                    mybir.AluOpType.min,
                ).then_inc(v_srv, 1)
                if i == NBB - 1:
                    # phase M of this rep
                    for k in range(NCB):
                        gk = r * NCB + k
                        cw = _cw(k)
                        sl = slice((gk % 2) * CB, (gk % 2) * CB + cw)
                        vector.wait_ge(sc_h0, gk + 1)
                        vector.wait_ge(d_m[gk % 2], (gk // 2 + 1) * 32)
                        if gk >= 2:
                            vector.wait_ge(t_h1, gk - 1)  # s_h0 slot consumed
                        nc.vector.tensor_tensor(
                            s_h0[:, sl], s_h0p[:, sl], s_zg[:, sl],
                            mybir.AluOpType.add,
                        ).then_inc(v_h0, 1)
                        vector.wait_ge(v_h0, 2 * gk + 1)  # same-engine RAW
                        nc.vector.tensor_scalar_max(
                            s_h0[:, sl], s_h0[:, sl], 0.0
                        ).then_inc(v_h0, 1)

        @block.scalar
        def _(scalar):
            AF = mybir.ActivationFunctionType
            for r in range(reps):
                for i in range(NBB):
                    gi = r * NBB + i
                    bsl = slice(i * BB, i * BB + BB)
                    scalar.wait_ge(t_nsc, gi + 1)
                    if gi >= NBB:
                        # s_nscT consumed by zb-mm + vector min of prev rep
                        scalar.wait_ge(t_zbir, gi - NBB + 1)
                        scalar.wait_ge(v_srv, gi - NBB + 1)
                    nc.scalar.copy(s_nscT[:, bsl], p_nsc(gi)).then_inc(sc_nsc, 1)
                for i in range(NBB):
                    gi = r * NBB + i
                    bsl = slice(i * BB, i * BB + BB)
                    scalar.wait_ge(t_zbir, gi + 1)
                    if r >= 1 and i == 0:
                        scalar.wait_ge(t_h0, NCB * r)   # s_zb consumed
                        scalar.wait_ge(d_ir, 16 * r)    # s_ir dma'd out
                    pm = p_misc(gi)
                    nc.scalar.copy(s_ir[:, bsl], pm[0:1, :]).then_inc(sc_zbir, 1)
                    nc.scalar.copy(s_zb[:, bsl], pm[32:64, :]).then_inc(sc_zbir, 1)
                # skewed phase M: h0p(kk) | relu-h1(kk-1) | out(kk-2)
                for kk in range(NCB + 2):
                    if kk < NCB:
                        gk = r * NCB + kk
                        cw = _cw(kk)
                        sl = slice((gk % 2) * CB, (gk % 2) * CB + cw)
                        scalar.wait_ge(t_h0, gk + 1)
                        if gk >= 2:
                            scalar.wait_ge(v_h0, 2 * gk - 3)  # h0p slot free
                        nc.scalar.copy(s_h0p[:, sl], p_h0(gk)).then_inc(sc_h0, 1)
                    if 1 <= kk and kk - 1 < NCB:
                        k = kk - 1
                        gk = r * NCB + k
                        cw = _cw(k)
                        sl = slice((gk % 2) * CB, (gk % 2) * CB + cw)
                        scalar.wait_ge(t_h1, gk + 1)
                        nc.scalar.activation(
                            s_h1[:, sl], p_h1(gk), AF.Relu
                        ).then_inc(sc_h1, 1)
                    if 2 <= kk and kk - 2 < NCB:
                        k = kk - 2
                        gk = r * NCB + k
                        cw = _cw(k)
                        scalar.wait_ge(t_o, gk + 1)
                        if gk >= 4:
                            scalar.wait_ge(d_out[gk % 4], ((gk - 4) // 4 + 1) * 16)
                        osl = slice((gk % 4) * CB, (gk % 4) * CB + cw)
                        nc.scalar.copy(s_o4[:, osl], p_o(gk)).then_inc(sc_o, 1)

    return nc


_NC = {}


def _get_nc(reps=1):
    if reps not in _NC:
        _NC[reps] = _build(reps)
    return _NC[reps]


def _prep_core(obs, ac, la, W0, zg_tab):
    """Host-side layout prep for one core's batch slice (all numpy)."""
    bc = obs.shape[0]
    out = {}
    # acT: [405, bc] padded to [512, bc] -> [4, NBB, 128, BB]
    acT = np.zeros((512, bc), np.float32)
    acT[:405] = ac.reshape(bc, 405).T
    out["acT"] = np.ascontiguousarray(
        acT.reshape(4, 128, NBB, BB).transpose(0, 2, 1, 3)
    ).astype(BF16)
    # demT: [NBB, 81, BB]
    dem = obs[:, S:2 * S].T  # [81, bc]
    out["demT"] = np.ascontiguousarray(
        dem.reshape(S, NBB, BB).transpose(1, 0, 2)
    ).astype(BF16)
    # zg: [128, NG] stacked by agent class, then col-blocked [NCB, 128, CB]
    loc = obs[:, 2 * S:2 * S + N_AG].astype(np.int64)  # [bc, 20]
    zst = np.empty((128, bc * 5), np.float32)
    for j in range(4):
        lj = loc[:, j::4].reshape(-1)            # cols (b, g)
        zst[32 * j:32 * j + 32] = zg_tab[lj].T   # [32, bc*5]
    zstp = np.zeros((128, NGP), np.float32)
    zstp[:, :bc * 5] = zst
    out["zgb"] = np.ascontiguousarray(
        zstp.reshape(128, NCB, CB).transpose(1, 0, 2)
    ).astype(BF16)
    # rhs0: rows 0-19 la packed, rows 20-24 g-onehot; [NCB, 25, CB]
    r0 = np.zeros((25, NGP), np.float32)
    for j in range(4):
        r0[5 * j:5 * j + 5, :bc * 5] = (
            la[:, j::4, :].transpose(2, 0, 1).reshape(5, bc * 5)
        )
    r0[20:25, :bc * 5] = np.tile(np.eye(5, dtype=np.float32), (1, bc))
    out["rhs0"] = np.ascontiguousarray(
        r0.reshape(25, NCB, CB).transpose(1, 0, 2)
    ).astype(BF16)
    return out


def kernel(obs, action_count, local_actions, W0, W1, W2, b2):
    obs = np.asarray(obs, np.float32)
    action_count = np.asarray(action_count, np.float32)
    local_actions = np.asarray(local_actions, np.float32)
    W0 = np.asarray(W0, np.float32)
    W1 = np.asarray(W1, np.float32)
    W2 = np.asarray(W2, np.float32)
    b2 = np.asarray(b2, np.float32)

    # ---- weight repacking (shared across cores) ----
    W0a, W0b = W0[0:S], W0[S:2 * S]           # [81,32], [81,32]
    W0c, W0d = W0[2 * S:2 * S + A], W0[2 * S + A:]  # [5,32], [20,32]
    inflow = _grid_inflow()
    wIN = np.zeros((512, S), np.float32)
    wIN[:405] = inflow
    wIN = wIN.reshape(4, 128, S).astype(BF16)
    w0le = np.zeros((25, 128), np.float32)
    for j in range(4):
        w0le[5 * j:5 * j + 5, 32 * j:32 * j + 32] = W0c
        for e in range(5):
            w0le[20 + e, 32 * j:32 * j + 32] = W0d[4 * e + j]
    wREP = np.tile(np.eye(32, dtype=np.float32), (1, 4))
    w1b = np.zeros((128, 128), np.float32)
    for j in range(4):
        w1b[32 * j:32 * j + 32, 32 * j:32 * j + 32] = W1
    w2i = np.zeros((128, 4), np.float32)
    for j in range(4):
        w2i[32 * j:32 * j + 32, j] = W2[:, 0]
    consts = {
        "wIN": wIN,
        "wA": W0a.astype(BF16),
        "w0le": w0le.astype(BF16),
        "wREP": wREP.astype(BF16),
        "w1": w1b.astype(BF16),
        "w2i": w2i.astype(BF16),
        "ones81": np.ones((S, 1), np.float32).astype(BF16),
    }

    in_maps = []
    for c in range(NCORES):
        bsl = slice(c * BC, (c + 1) * BC)
        m = _prep_core(
            obs[bsl], action_count[bsl], local_actions[bsl], W0, W0b
        )
        m.update(consts)
        in_maps.append(m)

    nc = _get_nc()
    res = run_bass_kernel_spmd(nc, in_maps, list(range(NCORES)))
    global _LAST
    _LAST = res

    out = np.empty((B, N_AG), np.float32)
    for c in range(NCORES):
        r = res.results[c]
        o4 = r["out4"].transpose(1, 0, 2).reshape(4, NGP)[:, :NG]  # [4, (b,g)]
        o4 = o4.reshape(4, BC, 5)
        ob = o4.transpose(1, 2, 0).reshape(BC, N_AG)  # out[b, 4g+j]
        ob += r["irO"][0][:, None] + b2[0]
        out[c * BC:(c + 1) * BC] = ob
    return out



# revision 8
# speedup vs baseline: 1.2233x; 1.2233x over previous
"""Trainium2 Bass kernel for DecGridDeepVPN (gnn_message_passing), 8-core SPMD.

Math (per batch row b, agents n=0..19):
  nsc[b]  = action_count[b].reshape(405) @ INFLOW            # [81]
  ir[b]   = sum_d min(nsc[b,d], demand[b,d])                 # scalar
  feat    = [nsc | onehot81(loc[b,n]) | la[b,n] | onehot20(n)]   # 187
  h0 = relu(feat @ W0); h1 = relu(h0 @ W1); out[b,n] = h1@W2 + b2 + ir[b]

Sharding: pure data-parallel over batch (4096 rows/core). Feature-major on
device (features on partitions, batch streaming on the free axis). MLP columns
are (b, g) pairs, g = n//4; agent classes j = n%4 occupy four 32-row partition
blocks, so layers run as block-diagonal matmuls with K=M=128.

Layer-0 decomposition (W0 = [W0a | W0b | W0c | W0d] over the feature concat):
  zbase = W0a.T @ nscT        broadcast over agents via a 0-step rhs AP (wREP)
  zg    = W0b[loc] + W0d[n]   embedding lookup + agent-onehot const, both
                              resolved as host-side layout prep, streamed bf16,
                              accumulated into PSUM via an identity matmul
  la term                     one K=20 block-diagonal matmul (w0le)
ir (+b2) is reduced on device (ones-vector matmul) and added in fp32 on the
host during unsharding.

All streamed operands are bf16; PSUM accumulation is fp32; MLP outputs return
bf16 (the fp32 ir term dominates the output scale, so bf16 MLP error is ~1e-4
relative). DMAs are batched to >=80KB so descriptor overhead stays small.
"""

from contextlib import ExitStack
import dataclasses as _dc

import numpy as np
import ml_dtypes

import concourse.bass as bass
import concourse.mybir as mybir
from concourse.bass_utils import run_bass_kernel_spmd

BF16 = ml_dtypes.bfloat16
F32 = mybir.dt.float32
BF = mybir.dt.bfloat16

S = 81          # grid states
GRID = 9
N_AG = 20       # agents
A = 5           # actions
B = 32768
NCORES = 8
BC = B // NCORES      # 4096 batch rows per core
NBB = 8               # b-blocks (phase N)
BB = BC // NBB        # 512
NG = BC * 5           # 20480 (b, g) columns per core
CB = 510              # col-block: multiple of 5, fits one PSUM bank
NCB = (NG + CB - 1) // CB   # 41 (last block = 80 cols)
GRP = 4               # col-blocks per zg/r0 DMA group
NGRP = (NCB + GRP - 1) // GRP   # 11
GCB = GRP * CB        # 2040 cols per DMA group

# wALL column offsets
_WIN = [0, 81, 162, 243]
_WA = 324
_W0LE = 356
_WREP = 484
_W1 = 612
_W2I = 740
_ONES = 744
_I128 = 745
WCOLS = 873

# output DMA chunks (block ranges)
OUT_CHUNKS = [(0, 11), (11, 22), (22, 33), (33, NCB)]


def _grid_inflow():
    moves = [(0, 0), (-1, 0), (1, 0), (0, -1), (0, 1)]
    inflow = np.zeros((S * A, S), np.float32)
    for s in range(S):
        r, c = divmod(s, GRID)
        for a, (dr, dc) in enumerate(moves):
            nr, nc_ = r + dr, c + dc
            d = nr * GRID + nc_ if (0 <= nr < GRID and 0 <= nc_ < GRID) else s
            inflow[s * A + a, d] = 1.0
    return inflow


def _cw(k):
    return min(CB, NG - k * CB)


def _build():
    nc = bass.Bass()

    wALL = nc.declare_dram_parameter("wALL", [128, WCOLS], BF, isOutput=False)
    acT = nc.declare_dram_parameter("acT", [NBB, 128, 4, BB], BF, isOutput=False)
    demT = nc.declare_dram_parameter("demT", [S, BC], BF, isOutput=False)
    zgb = nc.declare_dram_parameter("zgb", [NGRP, 128, GCB], BF, isOutput=False)
    r0b = nc.declare_dram_parameter("r0b", [NGRP, 20, GCB], BF, isOutput=False)
    outb = nc.declare_dram_parameter("outb", [4, NG], BF, isOutput=True)
    irO = nc.declare_dram_parameter("irO", [1, BC], F32, isOutput=True)

    ctx = ExitStack()
    with ctx:
        s_w = ctx.enter_context(nc.sbuf_tensor([128, WCOLS], BF))
        s_ac = ctx.enter_context(nc.sbuf_tensor([128, 2 * 4 * BB], BF))
        s_dem = ctx.enter_context(nc.sbuf_tensor([S, BC], BF))
        s_nscT = ctx.enter_context(nc.sbuf_tensor([S, BC], BF))
        s_srv = ctx.enter_context(nc.sbuf_tensor([S, 2 * BB], BF))
        s_zb = ctx.enter_context(nc.sbuf_tensor([32, BC], BF))
        s_ir = ctx.enter_context(nc.sbuf_tensor([1, BC], F32))
        s_zg = ctx.enter_context(nc.sbuf_tensor([128, 2 * GCB], BF))
        s_r0 = ctx.enter_context(nc.sbuf_tensor([20, 2 * GCB], BF))
        s_h0 = ctx.enter_context(nc.sbuf_tensor([128, 2 * CB], BF))
        s_h1 = ctx.enter_context(nc.sbuf_tensor([128, 2 * CB], BF))
        s_out = ctx.enter_context(nc.sbuf_tensor([4, NG], BF))
        psum = ctx.enter_context(nc.psum_tensor([128, 4096], F32))

        d_w = ctx.enter_context(nc.semaphore())
        d_dem = ctx.enter_context(nc.semaphore())
        d_ac = [ctx.enter_context(nc.semaphore(f"d_ac{i}")) for i in range(2)]
        d_m = [ctx.enter_context(nc.semaphore(f"d_m{i}")) for i in range(2)]
        d_out = ctx.enter_context(nc.semaphore())
        t_nsc = ctx.enter_context(nc.semaphore())
        t_zbir = ctx.enter_context(nc.semaphore())
        t_h0 = ctx.enter_context(nc.semaphore())
        t_h1 = ctx.enter_context(nc.semaphore())
        t_o = ctx.enter_context(nc.semaphore())
        v_srv = ctx.enter_context(nc.semaphore())
        v_zb = ctx.enter_context(nc.semaphore())
        v_h0 = ctx.enter_context(nc.semaphore())
        sc_nsc = ctx.enter_context(nc.semaphore())
        sc_ir = ctx.enter_context(nc.semaphore())
        sc_h1 = ctx.enter_context(nc.semaphore())
        sc_o = ctx.enter_context(nc.semaphore())
        block = ctx.enter_context(nc.Block())

        # PSUM bank map: 0/1 nsc|h0, 2/3 ir|h1, 4/5 zb|out
        def p_nsc(i):
            return psum[0:S, (i % 2) * 512:(i % 2) * 512 + BB]

        def p_ir(i):
            return psum[0:1, 1024 + (i % 2) * 512:1024 + (i % 2) * 512 + BB]

        def p_zb(i):
            return psum[0:32, 2048 + (i % 2) * 512:2048 + (i % 2) * 512 + BB]

        def p_h0(k):
            return psum[:, (k % 2) * 512:(k % 2) * 512 + _cw(k)]

        def p_h1(k):
            return psum[:, 1024 + (k % 2) * 512:1024 + (k % 2) * 512 + _cw(k)]

        def p_o(k):
            return psum[0:4, 2048 + (k % 2) * 512:2048 + (k % 2) * 512 + _cw(k)]

        def m_sl(k):
            # slice of s_zg/s_r0 for col-block k
            return slice(((k // GRP) % 2) * GCB + (k % GRP) * CB,
                         ((k // GRP) % 2) * GCB + (k % GRP) * CB + _cw(k))

        @block.sync
        def _(sync):
            sync.dma_start(s_w[:, :], wALL[:, :]).then_inc(d_w, 16)
            sync.dma_start(s_dem[:, :], demT[:, :]).then_inc(d_dem, 16)
            for i in range(NBB):
                if i >= 2:
                    sync.wait_ge(t_nsc, i - 1)  # s_ac slot free
                sl = slice((i % 2) * 4 * BB, (i % 2) * 4 * BB + 4 * BB)
                sync.dma_start(s_ac[:, sl], acT[i]).then_inc(d_ac[i % 2], 16)
            for g in range(NGRP):
                if g >= 2:
                    sync.wait_ge(t_h0, 4 * (g - 1))  # zg/r0 slot consumed
                sl = slice((g % 2) * GCB, (g % 2) * GCB + GCB)
                sync.dma_start(s_zg[:, sl], zgb[g]).then_inc(d_m[g % 2], 16)
                sync.dma_start(s_r0[:, sl], r0b[g]).then_inc(d_m[g % 2], 16)

        @block.gpsimd
        def _(gpsimd):
            for (k0, k1) in OUT_CHUNKS:
                gpsimd.wait_ge(sc_o, k1)
                c0, c1 = k0 * CB, k0 * CB + sum(_cw(k) for k in range(k0, k1))
                gpsimd.dma_start(outb[:, c0:c1], s_out[:, c0:c1]).then_inc(
                    d_out, 16)
            gpsimd.wait_ge(sc_ir, NBB)
            gpsimd.dma_start(irO[:, :], s_ir[:, :]).then_inc(d_out, 16)
            gpsimd.wait_ge(d_out, (len(OUT_CHUNKS) + 1) * 16)

        @block.tensor
        def _(tensor):
            tensor.wait_ge(d_w, 16)
            # ---- phase N: nscT ----
            for i in range(NBB):
                tensor.wait_ge(d_ac[i % 2], (i // 2 + 1) * 16)
                if i >= 2:
                    tensor.wait_ge(sc_nsc, i - 1)  # psum nsc bank free
                base = (i % 2) * 4 * BB
                for c in range(4):
                    mm = nc.tensor.matmul(
                        p_nsc(i), s_w[0:128, _WIN[c]:_WIN[c] + S],
                        s_ac[:, base + c * BB:base + (c + 1) * BB],
                        start=(c == 0), stop=(c == 3),
                    )
                mm.then_inc(t_nsc, 1)
            # ---- phase N: ir reduce + zbase ----
            for i in range(NBB):
                tensor.wait_ge(v_srv, i + 1)
                if i >= 2:
                    tensor.wait_ge(sc_ir, i - 1)   # ir bank free
                    tensor.wait_ge(v_zb, i - 1)    # zb bank free
                nc.tensor.matmul(
                    p_ir(i), s_w[0:S, _ONES:_ONES + 1],
                    s_srv[:, (i % 2) * BB:(i % 2) * BB + BB],
                    start=True, stop=True,
                ).then_inc(t_zbir, 1)
                nc.tensor.matmul(
                    p_zb(i), s_w[0:S, _WA:_WA + 32],
                    s_nscT[:, i * BB:(i + 1) * BB],
                    start=True, stop=True,
                ).then_inc(t_zbir, 1)
            # ---- phase M, software-pipelined ----
            tensor.wait_ge(v_zb, NBB)    # s_zb complete; zb banks free
            tensor.wait_ge(sc_nsc, NBB)  # banks 0-1 free
            tensor.wait_ge(sc_ir, NBB)   # banks 2-3 free
            for kk in range(NCB + 2):
                if kk < NCB:
                    k = kk
                    cw = _cw(k)
                    g = k // GRP
                    tensor.wait_ge(d_m[g % 2], (g // 2 + 1) * 32)
                    if k >= 2:
                        tensor.wait_ge(v_h0, k - 1)  # p_h0 bank free
                    sl = m_sl(k)
                    nc.tensor.matmul(
                        p_h0(k), s_w[0:20, _W0LE:_W0LE + 128],
                        s_r0[:, sl], start=True, stop=False,
                    )
                    b0 = (k * CB) // 5
                    nb = cw // 5
                    zb_ap = s_zb[:, b0:b0 + nb]
                    zb_ap = _dc.replace(
                        zb_ap, ap=[zb_ap.ap[0], [1, nb], [0, 5]])
                    nc.tensor.matmul(
                        p_h0(k), s_w[0:32, _WREP:_WREP + 128],
                        zb_ap, start=False, stop=False,
                    )
                    nc.tensor.matmul(
                        p_h0(k), s_w[0:128, _I128:_I128 + 128],
                        s_zg[:, sl], start=False, stop=True,
                    ).then_inc(t_h0, 1)
                if 1 <= kk and kk - 1 < NCB:
                    k = kk - 1
                    cw = _cw(k)
                    tensor.wait_ge(v_h0, k + 1)      # s_h0 ready
                    if k >= 2:
                        tensor.wait_ge(sc_h1, k - 1)  # p_h1 bank free
                    nc.tensor.matmul(
                        p_h1(k), s_w[0:128, _W1:_W1 + 128],
                        s_h0[:, (k % 2) * CB:(k % 2) * CB + cw],
                        start=True, stop=True,
                    ).then_inc(t_h1, 1)
                if 2 <= kk and kk - 2 < NCB:
                    k = kk - 2
                    cw = _cw(k)
                    tensor.wait_ge(sc_h1, k + 1)     # s_h1 ready
                    if k >= 2:
                        tensor.wait_ge(sc_o, k - 1)  # p_o bank free
                    nc.tensor.matmul(
                        p_o(k), s_w[0:128, _W2I:_W2I + 4],
                        s_h1[:, (k % 2) * CB:(k % 2) * CB + cw],
                        start=True, stop=True,
                    ).then_inc(t_o, 1)

        @block.vector
        def _(vector):
            vector.wait_ge(d_dem, 16)
            for i in range(NBB):
                vector.wait_ge(sc_nsc, i + 1)
                if i >= 2:
                    vector.wait_ge(t_zbir, 2 * (i - 2) + 1)  # s_srv slot free
                nc.vector.tensor_tensor(
                    s_srv[:, (i % 2) * BB:(i % 2) * BB + BB],
                    s_nscT[:, i * BB:(i + 1) * BB],
                    s_dem[:, i * BB:(i + 1) * BB],
                    mybir.AluOpType.min,
                ).then_inc(v_srv, 1)
                vector.wait_ge(t_zbir, 2 * i + 2)
                nc.vector.tensor_copy(
                    s_zb[:, i * BB:(i + 1) * BB], p_zb(i)
                ).then_inc(v_zb, 1)
            for k in range(NCB):
                cw = _cw(k)
                vector.wait_ge(t_h0, k + 1)
                if k >= 2:
                    vector.wait_ge(t_h1, k - 1)  # s_h0 slot free
                nc.vector.tensor_scalar_max(
                    s_h0[:, (k % 2) * CB:(k % 2) * CB + cw], p_h0(k), 0.0
                ).then_inc(v_h0, 1)

        @block.scalar
        def _(scalar):
            AF = mybir.ActivationFunctionType
            for i in range(NBB):
                scalar.wait_ge(t_nsc, i + 1)
                nc.scalar.copy(
                    s_nscT[:, i * BB:(i + 1) * BB], p_nsc(i)
                ).then_inc(sc_nsc, 1)
            for i in range(NBB):
                scalar.wait_ge(t_zbir, 2 * i + 1)
                nc.scalar.copy(
                    s_ir[:, i * BB:(i + 1) * BB], p_ir(i)
                ).then_inc(sc_ir, 1)
            # skewed phase M: relu-h1(kk-1) | out(kk-2)
            for kk in range(1, NCB + 2):
                if kk - 1 < NCB:
                    k = kk - 1
                    cw = _cw(k)
                    scalar.wait_ge(t_h1, k + 1)
                    nc.scalar.activation(
                        s_h1[:, (k % 2) * CB:(k % 2) * CB + cw], p_h1(k),
                        AF.Relu,
                    ).then_inc(sc_h1, 1)
                if 2 <= kk and kk - 2 < NCB:
                    k = kk - 2
                    cw = _cw(k)
                    scalar.wait_ge(t_o, k + 1)
                    nc.scalar.copy(
                        s_out[:, k * CB:k * CB + cw], p_o(k)
                    ).then_inc(sc_o, 1)

    return nc


_NC = {}


def _get_nc():
    if "nc" not in _NC:
        _NC["nc"] = _build()
    return _NC["nc"]


def _prep_core(obs, ac, la, zg_tab, w0d):
    """Host-side layout prep for one core's batch slice (all numpy)."""
    bc = obs.shape[0]
    out = {}
    # acT: [405, bc] padded to [512, bc] -> [NBB, 128, 4, BB]
    acT = np.zeros((512, bc), np.float32)
    acT[:405] = ac.reshape(bc, 405).T
    out["acT"] = np.ascontiguousarray(
        acT.reshape(4, 128, NBB, BB).transpose(2, 1, 0, 3)
    ).astype(BF16)
    # demT: [81, bc]
    out["demT"] = np.ascontiguousarray(obs[:, S:2 * S].T).astype(BF16)
    # zg: [128, NG] stacked by agent class (+ agent-onehot const folded),
    # then grouped [NGRP, 128, GCB]
    loc = obs[:, 2 * S:2 * S + N_AG].astype(np.int64)  # [bc, 20]
    zst = np.empty((128, bc * 5), np.float32)
    for j in range(4):
        lj = loc[:, j::4].reshape(-1)            # cols (b, g), g fastest
        zj = zg_tab[lj] + np.tile(w0d[j::4], (bc, 1))
        zst[32 * j:32 * j + 32] = zj.T           # [32, bc*5]
    zstp = np.zeros((128, NGRP * GCB), np.float32)
    zstp[:, :bc * 5] = zst
    out["zgb"] = np.ascontiguousarray(
        zstp.reshape(128, NGRP, GCB).transpose(1, 0, 2)
    ).astype(BF16)
    # r0: rows 5j+a = la[b, 4g+j, a]; [NGRP, 20, GCB]
    r0 = np.zeros((20, NGRP * GCB), np.float32)
    for j in range(4):
        r0[5 * j:5 * j + 5, :bc * 5] = (
            la[:, j::4, :].transpose(2, 0, 1).reshape(5, bc * 5)
        )
    out["r0b"] = np.ascontiguousarray(
        r0.reshape(20, NGRP, GCB).transpose(1, 0, 2)
    ).astype(BF16)
    return out


def kernel(obs, action_count, local_actions, W0, W1, W2, b2):
    obs = np.asarray(obs, np.float32)
    action_count = np.asarray(action_count, np.float32)
    local_actions = np.asarray(local_actions, np.float32)
    W0 = np.asarray(W0, np.float32)
    W1 = np.asarray(W1, np.float32)
    W2 = np.asarray(W2, np.float32)
    b2 = np.asarray(b2, np.float32)

    # ---- weight repacking (shared across cores) ----
    W0a, W0b = W0[0:S], W0[S:2 * S]                  # [81,32], [81,32]
    W0c, W0d = W0[2 * S:2 * S + A], W0[2 * S + A:]   # [5,32], [20,32]
    inflow = _grid_inflow()
    wIN = np.zeros((512, S), np.float32)
    wIN[:405] = inflow
    w0le = np.zeros((20, 128), np.float32)
    for j in range(4):
        for a in range(A):
            w0le[5 * j + a, 32 * j:32 * j + 32] = W0c[a]
    wALL = np.zeros((128, WCOLS), np.float32)
    for c in range(4):
        wALL[:, _WIN[c]:_WIN[c] + S] = wIN[128 * c:128 * (c + 1)]
    wALL[0:S, _WA:_WA + 32] = W0a
    wALL[0:20, _W0LE:_W0LE + 128] = w0le
    wALL[0:32, _WREP:_WREP + 128] = np.tile(np.eye(32, dtype=np.float32),
                                            (1, 4))
    for j in range(4):
        wALL[32 * j:32 * j + 32, _W1 + 32 * j:_W1 + 32 * j + 32] = W1
        wALL[32 * j:32 * j + 32, _W2I + j] = W2[:, 0]
    wALL[0:S, _ONES] = 1.0
    wALL[:, _I128:_I128 + 128] = np.eye(128, dtype=np.float32)
    wALL_bf = wALL.astype(BF16)

    in_maps = []
    for c in range(NCORES):
        bsl = slice(c * BC, (c + 1) * BC)
        m = _prep_core(obs[bsl], action_count[bsl], local_actions[bsl],
                       W0b, W0d)
        m["wALL"] = wALL_bf
        in_maps.append(m)

    nc = _get_nc()
    res = run_bass_kernel_spmd(nc, in_maps, list(range(NCORES)))
    global _LAST
    _LAST = res

    out = np.empty((B, N_AG), np.float32)
    for c in range(NCORES):
        r = res.results[c]
        o4 = np.asarray(r["outb"], np.float32)        # [4, NG], cols (b,g)
        ob = o4.reshape(4, BC, 5).transpose(1, 2, 0).reshape(BC, N_AG)
        ob += np.asarray(r["irO"], np.float32)[0][:, None] + b2[0]
        out[c * BC:(c + 1) * BC] = ob
    return out


# revision 18
# speedup vs baseline: 1.6571x; 1.3546x over previous
"""Trainium2 Bass kernel for DecGridDeepVPN (gnn_message_passing), 8-core SPMD.

Math (per batch row b, agents n=0..19):
  nsc[b]  = action_count[b].reshape(405) @ INFLOW            # [81]
  ir[b]   = sum_d min(nsc[b,d], demand[b,d])                 # scalar
  feat    = [nsc | onehot81(loc[b,n]) | la[b,n] | onehot20(n)]   # 187
  h0 = relu(feat @ W0); h1 = relu(h0 @ W1); out[b,n] = h1@W2 + b2 + ir[b]

Sharding: pure data-parallel over batch (4096 rows/core). Feature-major on
device (features on partitions, batch streaming on the free axis). MLP columns
are (g, b) pairs ordered g-major, g = n//4; agent classes j = n%4 occupy four
32-row partition blocks, so layers run as block-diagonal matmuls.

Every matmul is padded to K=128 / M=128 / N=512: the PE clock-gate (HAM) only
un-throttles to 2.4 GHz when the array is near-fully active, and a single
small-K matmul in the stream keeps the whole kernel at the cold 1.2 GHz rate
(measured: mixed-K chains never warm up; all-128 chains run 2x faster).

Layer-0 decomposition (W0 = [W0a | W0b | W0c | W0d] over the feature concat):
  zg    = W0b[loc] + W0d[n]   embedding lookup + agent-onehot const resolved
                              as host-side layout prep, streamed bf16, added
                              into PSUM via an identity matmul (K=128)
  zbase = W0a.T @ nscT        computed per-b, then replicated to the five
                              g-regions of an SBUF rhs buffer by SBUF-to-SBUF
                              DMA; consumed as rows 32-63 of the la matmul
  la term + zbase             one zero-padded K=128 matmul (rows 0-19 W0c
                              blocks, rows 32-63 identity-tiled)
ir (+b2) is reduced on device (ones-vector matmul) and added in fp32 on the
host during unsharding.

All streamed operands are bf16; PSUM accumulation is fp32; MLP outputs return
bf16 (the fp32 ir term dominates the output scale). DMAs are batched to
>=0.25MB so descriptor overhead stays small.
"""

from contextlib import ExitStack

import numpy as np
import ml_dtypes

import concourse.bass as bass
import concourse.mybir as mybir
from concourse.bass_utils import run_bass_kernel_spmd

BF16 = ml_dtypes.bfloat16
F32 = mybir.dt.float32
BF = mybir.dt.bfloat16

S = 81          # grid states
GRID = 9
N_AG = 20       # agents
A = 5           # actions
B = 32768
NCORES = 8
BC = B // NCORES      # 4096 batch rows per core
NBB = 8               # b-blocks (phase N)
BB = BC // NBB        # 512
NG = BC * 5           # 20480 (g, b) columns per core
CB = 512              # col-block, one PSUM bank
NCB = NG // CB        # 40
ZCH = 10              # col-blocks per zg DMA chunk
NZC = NCB // ZCH      # 4

# wALL column offsets (all weights zero-padded to [128, 128])
_WIN = [0, 128, 256, 384]
_ONESP = 512
_WAP = 640
_RZW = 768
_I128 = 896
_W1P = 1024
_W2IP = 1152
WCOLS = 1280

OUT_CHUNKS = [(0, 10), (10, 20), (20, 30), (30, 40)]


def _grid_inflow():
    moves = [(0, 0), (-1, 0), (1, 0), (0, -1), (0, 1)]
    inflow = np.zeros((S * A, S), np.float32)
    for s in range(S):
        r, c = divmod(s, GRID)
        for a, (dr, dc) in enumerate(moves):
            nr, nc_ = r + dr, c + dc
            d = nr * GRID + nc_ if (0 <= nr < GRID and 0 <= nc_ < GRID) else s
            inflow[s * A + a, d] = 1.0
    return inflow


def _build():
    nc = bass.Bass()

    wALL = nc.declare_dram_parameter("wALL", [128, WCOLS], BF, isOutput=False)
    acT = nc.declare_dram_parameter("acT", [NBB, 128, 4, BB], BF, isOutput=False)
    demT = nc.declare_dram_parameter("demT", [128, BC], BF, isOutput=False)
    zgb = nc.declare_dram_parameter("zgb", [128, NG], BF, isOutput=False)
    r0b = nc.declare_dram_parameter("r0b", [20, NG], BF, isOutput=False)
    outb = nc.declare_dram_parameter("outb", [4, NG], BF, isOutput=True)
    irO = nc.declare_dram_parameter("irO", [1, BC], F32, isOutput=True)

    ctx = ExitStack()
    with ctx:
        s_w = ctx.enter_context(nc.sbuf_tensor([128, WCOLS], BF))
        s_ac = ctx.enter_context(nc.sbuf_tensor([128, 4 * 4 * BB], BF))
        s_dem = ctx.enter_context(nc.sbuf_tensor([128, BC], BF))
        s_nscT = ctx.enter_context(nc.sbuf_tensor([128, BC], BF))
        s_srv = ctx.enter_context(nc.sbuf_tensor([128, 2 * BB], BF))
        s_ir = ctx.enter_context(nc.sbuf_tensor([1, BC], F32))
        s_zg = ctx.enter_context(nc.sbuf_tensor([128, NG], BF))
        s_rz = ctx.enter_context(nc.sbuf_tensor([128, NG], BF))
        s_h0 = ctx.enter_context(nc.sbuf_tensor([128, 4 * CB], BF))
        s_h1 = ctx.enter_context(nc.sbuf_tensor([128, 4 * CB], BF))
        s_out = ctx.enter_context(nc.sbuf_tensor([4, NG], BF))
        psum = ctx.enter_context(nc.psum_tensor([128, 4096], F32))

        d_w = ctx.enter_context(nc.semaphore())
        d_dem = ctx.enter_context(nc.semaphore())
        d_ac = [ctx.enter_context(nc.semaphore(f"d_ac{i}")) for i in range(4)]
        d_zg = [ctx.enter_context(nc.semaphore(f"d_zg{i}")) for i in range(NZC)]
        d_r0 = ctx.enter_context(nc.semaphore())
        d_zbb = ctx.enter_context(nc.semaphore())
        d_out = ctx.enter_context(nc.semaphore())
        t_nsc = ctx.enter_context(nc.semaphore())
        t_zbir = ctx.enter_context(nc.semaphore())
        t_h0 = ctx.enter_context(nc.semaphore())
        t_h1 = ctx.enter_context(nc.semaphore())
        t_o = ctx.enter_context(nc.semaphore())
        v_ms = ctx.enter_context(nc.semaphore())
        v_srv = ctx.enter_context(nc.semaphore())
        v_zb = ctx.enter_context(nc.semaphore())
        v_h0 = ctx.enter_context(nc.semaphore())
        sc_nsc = ctx.enter_context(nc.semaphore())
        sc_ir = ctx.enter_context(nc.semaphore())
        sc_h1 = ctx.enter_context(nc.semaphore())
        sc_o = ctx.enter_context(nc.semaphore())
        block = ctx.enter_context(nc.Block())

        # PSUM bank map: phase N nsc {0,1} ir {2,3} zb {4,5};
        # phase M h0 {0,1,2,3} h1 {4,5} out {6,7}
        def p_nsc(i):
            return psum[:, (i % 2) * 512:(i % 2) * 512 + BB]

        def p_irF(i):
            return psum[:, 1024 + (i % 2) * 512:1024 + (i % 2) * 512 + BB]

        def p_zbF(i):
            return psum[:, 2048 + (i % 2) * 512:2048 + (i % 2) * 512 + BB]

        def p_h0(k):
            return psum[:, (k % 4) * 512:(k % 4) * 512 + CB]

        def p_h1(k):
            return psum[:, 2048 + (k % 2) * 512:2048 + (k % 2) * 512 + CB]

        def p_o(k):
            return psum[:, 3072 + (k % 2) * 512:3072 + (k % 2) * 512 + CB]

        def ksl(k):
            return slice(k * CB, (k + 1) * CB)

        @block.sync
        def _(sync):
            sync.dma_start(s_w[:, :], wALL[:, :]).then_inc(d_w, 16)
            for i in range(4):
                sl = slice(i * 4 * BB, (i + 1) * 4 * BB)
                sync.dma_start(s_ac[:, sl], acT[i]).then_inc(d_ac[i % 4], 16)
            sync.dma_start(s_dem[:, :], demT[:, :]).then_inc(d_dem, 16)
            for c in range(NZC):
                sl = slice(c * ZCH * CB, (c + 1) * ZCH * CB)
                sync.dma_start(s_zg[:, sl], zgb[:, sl]).then_inc(d_zg[c], 16)
            for i in range(4, NBB):
                sync.wait_ge(t_nsc, i - 3)  # s_ac slot free
                sl = slice((i % 4) * 4 * BB, (i % 4) * 4 * BB + 4 * BB)
                sync.dma_start(s_ac[:, sl], acT[i]).then_inc(d_ac[i % 4], 16)
            sync.wait_ge(v_ms, 1)  # rz cleared before filling la rows
            sync.dma_start(s_rz[0:20, :], r0b[:, :]).then_inc(d_r0, 16)
            # replicate the zbase block to g-regions 1..4 (SBUF -> SBUF)
            sync.wait_ge(v_zb, NBB)
            for g in range(1, 5):
                sync.dma_start(
                    s_rz[32:64, g * BC:(g + 1) * BC], s_rz[32:64, 0:BC]
                ).then_inc(d_zbb, 16)

        @block.gpsimd
        def _(gpsimd):
            for (k0, k1) in OUT_CHUNKS:
                gpsimd.wait_ge(sc_o, k1)
                c0, c1 = k0 * CB, k1 * CB
                gpsimd.dma_start(outb[:, c0:c1], s_out[:, c0:c1]).then_inc(
                    d_out, 16)
            gpsimd.wait_ge(sc_ir, NBB)
            gpsimd.dma_start(irO[:, :], s_ir[:, :]).then_inc(d_out, 16)
            gpsimd.wait_ge(d_out, (len(OUT_CHUNKS) + 1) * 16)

        @block.tensor
        def _(tensor):
            tensor.wait_ge(d_w, 16)
            # ---- phase N: nscT ----
            for i in range(NBB):
                tensor.wait_ge(d_ac[i % 4], (i // 4 + 1) * 16)
                if i >= 2:
                    tensor.wait_ge(sc_nsc, i - 1)  # psum nsc bank free
                base = (i % 4) * 4 * BB
                for c in range(4):
                    mm = nc.tensor.matmul(
                        p_nsc(i), s_w[:, _WIN[c]:_WIN[c] + 128],
                        s_ac[:, base + c * BB:base + (c + 1) * BB],
                        start=(c == 0), stop=(c == 3),
                    )
                mm.then_inc(t_nsc, 1)
            # ---- phase N: ir reduce + zbase ----
            for i in range(NBB):
                tensor.wait_ge(v_srv, i + 1)
                if i >= 2:
                    tensor.wait_ge(sc_ir, i - 1)   # ir bank free
                    tensor.wait_ge(v_zb, i - 1)    # zb bank free
                nc.tensor.matmul(
                    p_irF(i), s_w[:, _ONESP:_ONESP + 128],
                    s_srv[:, (i % 2) * BB:(i % 2) * BB + BB],
                    start=True, stop=True,
                ).then_inc(t_zbir, 1)
                nc.tensor.matmul(
                    p_zbF(i), s_w[:, _WAP:_WAP + 128],
                    s_nscT[:, i * BB:(i + 1) * BB],
                    start=True, stop=True,
                ).then_inc(t_zbir, 1)
            # ---- phase M, software-pipelined: h0(kk) w1(kk-2) out(kk-3) ----
            tensor.wait_ge(sc_nsc, NBB)
            tensor.wait_ge(sc_ir, NBB)
            tensor.wait_ge(v_zb, NBB)
            tensor.wait_ge(v_ms, 1)      # rz pad rows zeroed
            tensor.wait_ge(d_r0, 16)
            for kk in range(NCB + 3):
                if kk < NCB:
                    k = kk
                    if k % ZCH == 0:
                        tensor.wait_ge(d_zg[k // ZCH], 16)
                    if k == 8:
                        tensor.wait_ge(d_zbb, 64)  # zbase replicas landed
                    if k >= 4:
                        tensor.wait_ge(v_h0, k - 3)  # p_h0 bank free
                    nc.tensor.matmul(
                        p_h0(k), s_w[:, _I128:_I128 + 128],
                        s_zg[:, ksl(k)], start=True, stop=False,
                    )
                    nc.tensor.matmul(
                        p_h0(k), s_w[:, _RZW:_RZW + 128],
                        s_rz[:, ksl(k)], start=False, stop=True,
                    ).then_inc(t_h0, 1)
                if 2 <= kk and kk - 2 < NCB:
                    k = kk - 2
                    tensor.wait_ge(v_h0, k + 1)      # s_h0 ready
                    if k >= 2:
                        tensor.wait_ge(sc_h1, k - 1)  # p_h1 bank free
                    nc.tensor.matmul(
                        p_h1(k), s_w[:, _W1P:_W1P + 128],
                        s_h0[:, (k % 4) * CB:(k % 4) * CB + CB],
                        start=True, stop=True,
                    ).then_inc(t_h1, 1)
                if 3 <= kk and kk - 3 < NCB:
                    k = kk - 3
                    tensor.wait_ge(sc_h1, k + 1)     # s_h1 ready
                    if k >= 2:
                        tensor.wait_ge(sc_o, k - 1)  # p_o bank free
                    nc.tensor.matmul(
                        p_o(k), s_w[:, _W2IP:_W2IP + 128],
                        s_h1[:, (k % 4) * CB:(k % 4) * CB + CB],
                        start=True, stop=True,
                    ).then_inc(t_o, 1)

        @block.vector
        def _(vector):
            # zero s_rz once (engine ops need 32-aligned start partitions, so
            # clear the whole tensor; the r0 DMA / zb evacs overwrite their
            # rows afterwards — r0b is gated on v_ms)
            nc.vector.memset(s_rz[:, :], 0.0).then_inc(v_ms, 1)
            vector.wait_ge(d_dem, 16)
            for i in range(NBB):
                vector.wait_ge(sc_nsc, i + 1)
                if i >= 2:
                    vector.wait_ge(t_zbir, 2 * (i - 2) + 1)  # s_srv slot free
                nc.vector.tensor_tensor(
                    s_srv[:, (i % 2) * BB:(i % 2) * BB + BB],
                    s_nscT[:, i * BB:(i + 1) * BB],
                    s_dem[:, i * BB:(i + 1) * BB],
                    mybir.AluOpType.min,
                ).then_inc(v_srv, 1)
                vector.wait_ge(t_zbir, 2 * i + 2)
                nc.vector.tensor_copy(
                    s_rz[32:64, i * BB:(i + 1) * BB], p_zbF(i)[32:64, :]
                ).then_inc(v_zb, 1)
            for k in range(NCB):
                vector.wait_ge(t_h0, k + 1)
                if k >= 4:
                    vector.wait_ge(t_h1, k - 3)  # s_h0 slot free
                nc.vector.tensor_scalar_max(
                    s_h0[:, (k % 4) * CB:(k % 4) * CB + CB], p_h0(k), 0.0
                ).then_inc(v_h0, 1)

        @block.scalar
        def _(scalar):
            AF = mybir.ActivationFunctionType
            for i in range(NBB):
                scalar.wait_ge(t_nsc, i + 1)
                nc.scalar.copy(
                    s_nscT[:, i * BB:(i + 1) * BB], p_nsc(i)
                ).then_inc(sc_nsc, 1)
            for i in range(NBB):
                scalar.wait_ge(t_zbir, 2 * i + 1)
                nc.scalar.copy(
                    s_ir[:, i * BB:(i + 1) * BB], p_irF(i)[0:1, :]
                ).then_inc(sc_ir, 1)
            # skewed phase M: relu-h1(kk-2) | out(kk-3)
            for kk in range(2, NCB + 3):
                if kk - 2 < NCB:
                    k = kk - 2
                    scalar.wait_ge(t_h1, k + 1)
                    if k >= 4:
                        scalar.wait_ge(t_o, k - 3)  # s_h1 slot free
                    nc.scalar.activation(
                        s_h1[:, (k % 4) * CB:(k % 4) * CB + CB], p_h1(k),
                        AF.Relu,
                    ).then_inc(sc_h1, 1)
                if 3 <= kk and kk - 3 < NCB:
                    k = kk - 3
                    scalar.wait_ge(t_o, k + 1)
                    nc.scalar.copy(
                        s_out[:, ksl(k)], p_o(k)[0:4, :]
                    ).then_inc(sc_o, 1)

    return nc


_NC = {}


def _get_nc():
    if "nc" not in _NC:
        _NC["nc"] = _build()
    return _NC["nc"]


def _prep_core(obs, ac, la, zg_tab, w0d):
    """Host-side layout prep for one core's batch slice (all numpy)."""
    bc = obs.shape[0]
    out = {}
    # acT: [405, bc] padded to [512, bc] -> [NBB, 128, 4, BB]
    acT = np.zeros((512, bc), np.float32)
    acT[:405] = ac.reshape(bc, 405).T
    out["acT"] = np.ascontiguousarray(
        acT.reshape(4, 128, NBB, BB).transpose(2, 1, 0, 3)
    ).astype(BF16)
    # demT: [128, bc], rows 81-127 zero
    dem = np.zeros((128, bc), np.float32)
    dem[:S] = obs[:, S:2 * S].T
    out["demT"] = dem.astype(BF16)
    # zg: [128, NG], cols (g, b) g-major; class j in rows 32j:32j+32;
    # includes the agent-onehot constant W0d[n]
    loc = obs[:, 2 * S:2 * S + N_AG].astype(np.int64)  # [bc, 20]
    zst = np.empty((128, NG), np.float32)
    for j in range(4):
        lj = loc[:, j::4].T.reshape(-1)          # cols (g, b), b fastest
        zj = zg_tab[lj] + np.repeat(w0d[j::4], bc, axis=0)
        zst[32 * j:32 * j + 32] = zj.T
    out["zgb"] = zst.astype(BF16)
    # r0: rows 5j+a = la[b, 4g+j, a]; [20, NG] cols (g, b)
    r0 = np.empty((20, NG), np.float32)
    for j in range(4):
        r0[5 * j:5 * j + 5] = (
            la[:, j::4, :].transpose(2, 1, 0).reshape(5, NG)
        )
    out["r0b"] = r0.astype(BF16)
    return out


def _build_wall(W0, W1, W2):
    """Pack all (zero-padded 128x128) stationary weights into one blob."""
    W0a = W0[0:S]
    W0c = W0[2 * S:2 * S + A]
    inflow = _grid_inflow()
    wIN = np.zeros((512, S), np.float32)
    wIN[:405] = inflow
    wALL = np.zeros((128, WCOLS), np.float32)
    for c in range(4):
        wALL[:, _WIN[c]:_WIN[c] + S] = wIN[128 * c:128 * (c + 1)]
    wALL[0:S, _ONESP] = 1.0
    wALL[0:S, _WAP + 32:_WAP + 64] = W0a
    for j in range(4):
        for a in range(A):
            wALL[5 * j + a, _RZW + 32 * j:_RZW + 32 * j + 32] = W0c[a]
        wALL[32:64, _RZW + 32 * j:_RZW + 32 * j + 32] = np.eye(
            32, dtype=np.float32)
        wALL[32 * j:32 * j + 32, _W1P + 32 * j:_W1P + 32 * j + 32] = W1
        wALL[32 * j:32 * j + 32, _W2IP + j] = W2[:, 0]
    wALL[:, _I128:_I128 + 128] = np.eye(128, dtype=np.float32)
    return wALL.astype(BF16)


def kernel(obs, action_count, local_actions, W0, W1, W2, b2):
    obs = np.asarray(obs, np.float32)
    action_count = np.asarray(action_count, np.float32)
    local_actions = np.asarray(local_actions, np.float32)
    W0 = np.asarray(W0, np.float32)
    W1 = np.asarray(W1, np.float32)
    W2 = np.asarray(W2, np.float32)
    b2 = np.asarray(b2, np.float32)

    wALL_bf = _build_wall(W0, W1, W2)
    W0b = W0[S:2 * S]
    W0d = W0[2 * S + A:]

    in_maps = []
    for c in range(NCORES):
        bsl = slice(c * BC, (c + 1) * BC)
        m = _prep_core(obs[bsl], action_count[bsl], local_actions[bsl],
                       W0b, W0d)
        m["wALL"] = wALL_bf
        in_maps.append(m)

    nc = _get_nc()
    res = run_bass_kernel_spmd(nc, in_maps, list(range(NCORES)))
    global _LAST
    _LAST = res

    out = np.empty((B, N_AG), np.float32)
    for c in range(NCORES):
        r = res.results[c]
        o4 = np.asarray(r["outb"], np.float32)        # [4, NG], cols (g, b)
        ob = o4.reshape(4, 5, BC).transpose(2, 1, 0).reshape(BC, N_AG)
        ob += np.asarray(r["irO"], np.float32)[0][:, None] + b2[0]
        out[c * BC:(c + 1) * BC] = ob
    return out
